# revision 13
# baseline (speedup 1.0000x reference)
"""
CSAM (channel self-attention) Trainium2 Bass kernel.

Computation (per batch b):
    q = x[b].reshape(C, N)                 # C=64, N=192*192=36864
    E = q @ q.T                            # [64, 64] channel gram
    A = softmax(rowmax(E) - E) over rows   # == softmax(-E) stabilized by rowmin
    out = A @ q
    res = x * (gamma * out) + x

Sharding: data-parallel over batch. 8 cores x 4 batches each; each core runs an
identical NEFF on its own batch slice (no collectives).

Layout: q lives in SBUF in the channel-interleaved flat layout p = 2c + h
(partition p holds q[c, 18432*h + j], i.e. x[b].flat reshaped [128, 18432]) so
loads/stores are flat contiguous DMAs. Pipeline (per batch, 6 column groups of
3072):
 - load group f32 -> cast once to a resident bf16 copy qbf (ACT);
 - PE transposes 128x128 chunks of qbf into PSUM (bursts of 8, two banks),
   dense PSUM->SBUF drains split between ACT and DVE;
 - energy: one matmul per chunk accumulates the interleaved gram
   Epp[p1,p2] = sum_n qT[n,p1] qT[n,p2]  (p = 2c+h; the h!=h' cross blocks are
   unused garbage, same moving cycles as two parity matmuls but half the
   instructions and no strided operands);
 - at batch end, E[c,d] = Epp[2c,2d] + Epp[2c+1,2d+1] is extracted with 4 tiny
   selector matmuls using Epp's symmetry, then softmax with gamma folded in and
   W = kron(A^T, I2) built on-chip;
 - out matmuls contract all 128 partitions: po = W^T @ qbf per 512-chunk;
   epilogue res = (po + 1) * qbf (DVE), stored flat per group via the gpsimd
   SWDGE queue (keeps store doorbells off the ACT/DVE critical path).
Phase1 of batch b is interleaved with phase2 of batch b-1 at group granularity
(phase1 first so transposes cover the softmax serial chain) so PE/ACT/DVE
always have work while DMA streams loads+stores.
"""

import sys

sys.path.insert(0, "/opt/trn_rl_repo")

import numpy as np

import concourse.bass as bass
import concourse.bacc as bacc
import concourse.tile as tile
from concourse import mybir
from concourse.bass_utils import run_bass_kernel_spmd
from concourse.masks import make_identity

N_CORES = 8
B_FULL, C, H, W = 32, 64, 192, 192
N = H * W                  # 36864
NH = N // 2                # 18432 flat-tile free size
B_PER = B_FULL // N_CORES  # 4 batches per core
NG = 6                     # column groups per batch
GW = NH // NG              # 3072 columns per group
TPG = GW // 128            # 24 transpose chunks per group
TSUB = 8                   # transpose burst size (one PSUM bank)
CHUNK = 512                # out-matmul free dim (one PSUM bank)
CPG = GW // CHUNK          # 6 out chunks per group

f32 = mybir.dt.float32
bf16 = mybir.dt.bfloat16

_CACHED_NC = None


def _build():
    nc = bacc.Bacc("TRN2", target_bir_lowering=False, debug=False)
    x_d = nc.dram_tensor("x", [B_PER, C, N], f32, kind="ExternalInput").ap()
    g_d = nc.dram_tensor("gamma", [1], f32, kind="ExternalInput").ap()
    o_d = nc.dram_tensor("out", [B_PER, C, N], f32, kind="ExternalOutput").ap()

    with tile.TileContext(nc) as tc:
        with (
            tc.tile_pool(name="const", bufs=1) as constp,
            tc.tile_pool(name="qf", bufs=4) as qfp,
            tc.tile_pool(name="qbf", bufs=3) as qbfp,
            tc.tile_pool(name="qT", bufs=4) as qtp,
            tc.tile_pool(name="res", bufs=2) as resp,
            tc.tile_pool(name="sm", bufs=2) as smp,
            tc.tile_pool(name="psE", bufs=2, space="PSUM") as psE,
            tc.tile_pool(name="psO", bufs=2, space="PSUM") as psO,
            tc.tile_pool(name="psT", bufs=2, space="PSUM") as psT,
            tc.tile_pool(name="psA", bufs=1, space="PSUM") as psA,
            tc.tile_pool(name="psW", bufs=1, space="PSUM") as psW,
        ):
            identb = constp.tile([128, 128], bf16)
            identf = constp.tile([128, 128], f32)
            make_identity(nc, identf[:])
            nc.vector.tensor_copy(identb[:], identf[:])
            g1 = constp.tile([1, 1], f32)
            nc.scalar.dma_start(g1[:], g_d[None, :])
            gb = constp.tile([128, 1], f32)
            nc.gpsimd.partition_broadcast(gb[:], g1[:])
            # selector constants: K2e[d, m] = 1 iff m == 2d; K2o: m == 2d+1
            K2e = constp.tile([64, 128], bf16)
            nc.gpsimd.memset(K2e[:], 0.0)
            nc.gpsimd.affine_select(
                out=K2e[:], in_=K2e[:],
                compare_op=mybir.AluOpType.not_equal,
                fill=1.0, base=0, pattern=[[-1, 128]], channel_multiplier=2,
            )
            K2o = constp.tile([64, 128], bf16)
            nc.gpsimd.memset(K2o[:], 0.0)
            nc.gpsimd.affine_select(
                out=K2o[:], in_=K2o[:],
                compare_op=mybir.AluOpType.not_equal,
                fill=1.0, base=1, pattern=[[-1, 128]], channel_multiplier=2,
            )
            # transposed selectors (f32, to pair with f32 Epp operands):
            # KeT[p, c] = 1 iff p == 2c; KoT: p == 2c + 1
            KeT = constp.tile([128, 64], f32)
            nc.gpsimd.memset(KeT[:], 0.0)
            nc.gpsimd.affine_select(
                out=KeT[:], in_=KeT[:],
                compare_op=mybir.AluOpType.not_equal,
                fill=1.0, base=0, pattern=[[-2, 64]], channel_multiplier=1,
            )
            KoT = constp.tile([128, 64], f32)
            nc.gpsimd.memset(KoT[:], 0.0)
            nc.gpsimd.affine_select(
                out=KoT[:], in_=KoT[:],
                compare_op=mybir.AluOpType.not_equal,
                fill=1.0, base=-1, pattern=[[-2, 64]], channel_multiplier=1,
            )

            def softmax_W(Epp):
                # E[c,d] = Epp[2c,2d] + Epp[2c+1,2d+1], via Epp's symmetry:
                #   Q_h = Epp^T KhT = Epp KhT  (Q_h[i, d] = Epp[2d+h, i])
                #   E   = sum_h KhT^T Q_h      (E[c, d] = Q_h[2c+h, d])
                Epp_sb = smp.tile([128, 128], f32, tag="Epp_sb")
                nc.vector.tensor_copy(Epp_sb[:], Epp[:])
                scr = psA.tile([128, 512], f32, tag="scr")
                Qe, Qo = scr[:, 0:64], scr[:, 64:128]
                Ecd = scr[0:64, 128:192]
                Zp0 = scr[0:64, 192:320]
                Zp1 = scr[0:64, 320:448]
                nc.tensor.matmul(Qe, Epp_sb[:], KeT[:], start=True, stop=True)
                nc.tensor.matmul(Qo, Epp_sb[:], KoT[:], start=True, stop=True)
                Qeo_sb = smp.tile([128, 128], f32, tag="Qeo_sb")
                nc.scalar.copy(Qeo_sb[:], scr[:, 0:128])
                nc.tensor.matmul(
                    Ecd, KeT[:], Qeo_sb[:, 0:64], start=True, stop=False
                )
                nc.tensor.matmul(
                    Ecd, KoT[:], Qeo_sb[:, 64:128], start=False, stop=True
                )
                # row-min-stabilized softmax of -E
                m = smp.tile([C, 1], f32, tag="m")
                nc.vector.tensor_reduce(
                    m[:], Ecd, axis=mybir.AxisListType.X, op=mybir.AluOpType.min
                )
                texp = smp.tile([C, C], f32, tag="texp")
                Z = smp.tile([C, 1], f32, tag="Z")
                nc.scalar.activation(
                    texp[:],
                    Ecd,
                    mybir.ActivationFunctionType.Exp,
                    bias=m[:],
                    scale=-1.0,
                    accum_out=Z[:],
                )
                r = smp.tile([C, 1], f32, tag="r")
                nc.vector.reciprocal(r[:], Z[:])
                # fold gamma into A so the epilogue is res = (out + 1) * x
                rg = smp.tile([C, 1], f32, tag="rg")
                nc.vector.tensor_tensor(rg[:], r[:], gb[0:64, :], mybir.AluOpType.mult)
                A = smp.tile([C, C], bf16, tag="A")
                nc.vector.tensor_scalar_mul(A[:], texp[:], rg[:])
                # W = kron(A^T, I2):  W[2d+h, 2c+h] = A[c, d]
                nc.tensor.matmul(Zp0, A[:], K2e[:], start=True, stop=True)
                nc.tensor.matmul(Zp1, A[:], K2o[:], start=True, stop=True)
                Zsb = smp.tile([C, 2, 128], bf16, tag="Zsb")
                nc.scalar.copy(Zsb[:, 0, :], Zp0)
                nc.scalar.copy(Zsb[:, 1, :], Zp1)
                Wpt = psW.tile([128, 128], f32, tag="Wp")
                Wp = Wpt[:]
                nc.tensor.matmul(Wp, K2e[:], Zsb[:, 0, :], start=True, stop=False)
                nc.tensor.matmul(Wp, K2o[:], Zsb[:, 1, :], start=False, stop=True)
                Wsb = smp.tile([128, 128], bf16, tag="Wsb")
                nc.scalar.copy(Wsb[:], Wp)
                return Wsb

            def phase1_group(b, g, qbf, Epp):
                # load f32 group, cast to resident bf16, transpose + energy
                xb = x_d[b].rearrange("c (h j) -> (c h) j", h=2)  # [128, 18432]
                qf = qfp.tile([128, GW], f32, tag="qf")
                nc.sync.dma_start(qf[:], xb[:, g * GW : (g + 1) * GW])
                SC = GW // 3  # sub-cast width: transposes start sooner
                for s in range(3):
                    nc.scalar.copy(
                        qbf[:, g * GW + s * SC : g * GW + (s + 1) * SC],
                        qf[:, s * SC : (s + 1) * SC],
                    )
                qts = []
                for sub in range(TPG // TSUB):
                    pq = psT.tile([128, TSUB, 128], bf16, tag="pq")
                    for ti in range(TSUB):
                        col = g * GW + (sub * TSUB + ti) * 128
                        nc.tensor.transpose(
                            pq[:, ti, :],
                            qbf[:, col : col + 128],
                            identb[:],
                        )
                    qT = qtp.tile([128, TSUB, 128], bf16, tag="qT")
                    nc.scalar.copy(qT[:, 0 : TSUB // 2, :], pq[:, 0 : TSUB // 2, :])
                    nc.vector.tensor_copy(
                        qT[:, TSUB // 2 : TSUB, :], pq[:, TSUB // 2 : TSUB, :]
                    )
                    qts.append(qT)
                for sub in range(TPG // TSUB):
                    for ti in range(TSUB):
                        t = g * TPG + sub * TSUB + ti
                        lr = qts[sub][:, ti, :]
                        nc.tensor.matmul(
                            Epp[:],
                            lr,
                            lr,
                            start=(t == 0),
                            stop=(t == NH // 128 - 1),
                        )

            def phase2_group(b, g, qbf, Wsb):
                # one column group: 6 out-matmuls + epilogue + flat store
                ob = o_d[b].rearrange("c (h j) -> (c h) j", h=2)
                res = resp.tile([128, GW], f32, tag="res")
                for i in range(CPG):
                    off = g * GW + i * CHUNK
                    po = psO.tile([128, CHUNK], f32, tag="po")
                    nc.tensor.matmul(
                        po[:],
                        Wsb[:],
                        qbf[:, off : off + CHUNK],
                        start=True,
                        stop=True,
                    )
                    nc.vector.scalar_tensor_tensor(
                        res[:, i * CHUNK : (i + 1) * CHUNK],
                        po[:],
                        1.0,
                        qbf[:, off : off + CHUNK],
                        mybir.AluOpType.add,
                        mybir.AluOpType.mult,
                    )
                nc.gpsimd.dma_start(ob[:, g * GW : (g + 1) * GW], res[:])

            # software pipeline, interleaved at column-group granularity.
            # softmax_W(b-1) is emitted after phase1(b, 0) and phase2(b-1, g)
            # after phase1(b, g+1): next-batch transpose/energy bursts cover
            # the serial softmax chain, and ready out-matmuls never queue
            # behind the not-yet-ready W build on the in-order PE queue.
            prevEpp = None
            prev = None
            for b in range(B_PER):
                qbf = qbfp.tile([128, NH], bf16, tag="qbf")
                Epp = psE.tile([128, 128], f32, tag="Epp")
                for g in range(NG):
                    phase1_group(b, g, qbf, Epp)
                    if b > 0:
                        if g == 0:
                            Wsb = softmax_W(prevEpp[1])
                            prev = (b - 1, prevEpp[0], Wsb)
                        else:
                            phase2_group(prev[0], g - 1, prev[1], prev[2])
                if b > 0:
                    phase2_group(prev[0], NG - 2, prev[1], prev[2])
                    phase2_group(prev[0], NG - 1, prev[1], prev[2])
                prevEpp = (qbf, Epp)
            Wsb = softmax_W(prevEpp[1])
            prev = (B_PER - 1, prevEpp[0], Wsb)
            for g in range(NG):
                phase2_group(prev[0], g, prev[1], prev[2])

    nc.compile()
    return nc


def _get_nc():
    global _CACHED_NC
    if _CACHED_NC is None:
        _CACHED_NC = _build()
    return _CACHED_NC


def kernel(x: np.ndarray, gamma: np.ndarray, _collect=None) -> np.ndarray:
    assert x.shape == (B_FULL, C, H, W) and x.dtype == np.float32
    nc = _get_nc()
    xr = np.ascontiguousarray(x.reshape(B_FULL, C, N), dtype=np.float32)
    gamma = np.ascontiguousarray(gamma, dtype=np.float32)
    in_maps = [
        {"x": xr[i * B_PER : (i + 1) * B_PER], "gamma": gamma}
        for i in range(N_CORES)
    ]
    r = run_bass_kernel_spmd(nc, in_maps, core_ids=list(range(N_CORES)))
    if _collect is not None:
        _collect.append(r)
    out = np.concatenate([r.results[i]["out"] for i in range(N_CORES)], axis=0)
    return out.reshape(B_FULL, C, H, W).astype(np.float32)
